# revision 1
# baseline (speedup 1.0000x reference)
"""Trainium2 Bass kernel for nn_Concat_Model_89343909692135.

Computes out[b,i,j] = sigmoid(w_b.x1[b,i] + w_a.x1[b,j] + bias) for
B=2, N=4096, F=320, distributed over 8 NeuronCores.

Sharding: core k handles batch b = k//4, row block m = k%4 (1024 rows).
Each core receives its batch's x1 rolled so its own 1024 rows come
first (the SPMD program is identical across cores; only data differs),
and writes its output block TRANSPOSED: out_t[j, i] with j = all 4096
(rolled) column nodes on the partition axis and i = the core's 1024
own rows on the free axis. The host un-rolls and transposes back.

Why transposed: the j-dependent term p_j = x1[j].w_a lands naturally
in partition layout from the DVE dot products and feeds the Sigmoid
activation's per-partition bias directly — no broadcast needed. Only
the i-dependent term p_i (1024 values) must be broadcast across
partitions, and that happens ONCE per core (PE transpose + masked
ones-matmul), not once per chunk.

Device program:
  - hoisted x1 loads (8 SWDGE DMAs) keep the DMA engines busy while
    compute ramps.
  - B_i[p, i] = p_i[i] + conv_b, built once: DVE dots for the own 8
    row tiles -> [128,8], PE transpose -> [8,128], mask with the 8x8
    identity into a block-diagonal [8,8,128], one K=8 ones-matmul per
    PSUM bank -> [128,1024], copied to SBUF with the conv_b add folded
    in.
  - per 128-j tile (32 total): DVE dot -> p_j tile [128,1], one
    Sigmoid activation out = sigmoid(B_i + bias=p_j) on ScalarE, one
    fully-contiguous 512 KB store on the sync HWDGE queue.
"""

import numpy as np

import concourse.bass as bass
import concourse.mybir as mybir
import concourse.tile as tile
from concourse import bass_utils

B = 2
N = 4096
F = 320
P = 128
N_CORES = 8
BLOCKS_PER_BATCH = N_CORES // B  # 4
ROWS_PER_CORE = N // BLOCKS_PER_BATCH  # 1024
ROW_TILES = ROWS_PER_CORE // P  # 8
COL_TILES = N // P  # 32
LOAD_GROUP = 4  # column tiles per load DMA
BANK = 512  # fp32 elements per PSUM bank


def _split_multiwait_instructions(nc):
    # The walrus build here only accepts one sem-wait per instruction.
    # Hoist extra waits onto preceding NoOps on the same engine queue;
    # in-order execution per engine makes this equivalent.
    seen_dma = False
    for fn in nc.m.functions:
        for bb in fn.blocks:
            new_list = []
            for ins in bb.instructions:
                # strip the all-engine ENTRY barrier (drain + EVSEM
                # butterfly before any real work): engines enter with
                # clean state (the exit sequence cleared sems) and all
                # real cross-engine deps are explicit Tile semaphores
                nm = type(ins).__name__
                if nm == "InstDMACopy":
                    seen_dma = True
                if not seen_dma and nm in ("InstDrain", "InstEventSemaphore"):
                    continue
                # drop the framework's unused const-tile memsets (the
                # verifier flags them as having no reader); they sit at
                # the head of the Pool queue and delay the first x1
                # load emission
                if (
                    type(ins).__name__ == "InstMemset"
                    and ins.outs
                    and getattr(ins.outs[0], "memref", "")
                    in (
                        "const-float32-0.0",
                        "const-float32-1.0",
                        "const-bfloat16-1.0",
                        "const-uint8-127",
                    )
                ):
                    continue
                si = getattr(ins, "sync_info", None)
                if si is not None and si.on_wait and len(si.on_wait) > 1:
                    waits = list(si.on_wait)
                    for i, w in enumerate(waits[:-1]):
                        nop = mybir.InstNoOp(
                            name=f"{ins.name}-w{i}",
                            ins=[],
                            outs=[],
                            engine=ins.engine,
                            sync_info=type(si)(on_wait=[w], on_update=[]),
                        )
                        new_list.append(nop)
                    si.on_wait = waits[-1:]
                new_list.append(ins)
            bb.instructions[:] = new_list


def _build_program(fixup=True):
    nc = bass.Bass("TRN2", debug=False, target_bir_lowering=False)
    f32 = mybir.dt.float32
    x_d = nc.dram_tensor("x1r", [N, F], f32, kind="ExternalInput").ap()
    w_d = nc.dram_tensor("conv_w", [2 * F], f32, kind="ExternalInput").ap()
    b_d = nc.dram_tensor("conv_b", [1], f32, kind="ExternalInput").ap()
    id_d = nc.dram_tensor("ident", [P, P], f32, kind="ExternalInput").ap()
    on_d = nc.dram_tensor("ones", [ROW_TILES, P], f32, kind="ExternalInput").ap()
    o_d = nc.dram_tensor("out", [N, ROWS_PER_CORE], f32, kind="ExternalOutput").ap()

    with tile.TileContext(nc) as tc:
        with (
            tc.tile_pool(name="singles", bufs=1) as singles,
            tc.tile_pool(name="xpool", bufs=1) as xpool,
            tc.tile_pool(name="small", bufs=2) as small,
            tc.tile_pool(name="outp", bufs=8) as outp,
            tc.tile_pool(name="psum", bufs=1, space="PSUM") as psum,
            tc.tile_pool(name="pst", bufs=1, space="PSUM") as pst,
        ):
            w_rep = singles.tile([P, 2 * F], f32)
            b_bcast = singles.tile([P, 1], f32)
            ident = singles.tile([P, P], f32)
            ones_k = singles.tile([ROW_TILES, P], f32)
            # w_b half first: it gates the very first p_i dot
            nc.sync.dma_start(
                out=w_rep[:, F : 2 * F], in_=w_d[F : 2 * F].partition_broadcast(P)
            )
            nc.sync.dma_start(
                out=w_rep[:, 0:F], in_=w_d[0:F].partition_broadcast(P)
            )
            nc.sync.dma_start(out=b_bcast, in_=b_d.partition_broadcast(P))
            nc.sync.dma_start(out=ident, in_=id_d)
            nc.sync.dma_start(out=ones_k, in_=on_d)
            w_a_rep = w_rep[:, 0:F]
            w_b_rep = w_rep[:, F : 2 * F]

            # warm-up: trigger the sigmoid ACT-table load (~2.7us on
            # real HW, invisible to the cost model) while x1 streams in
            warm = singles.tile([P, 1], f32)
            nc.scalar.activation(
                out=warm,
                in_=b_bcast,
                func=mybir.ActivationFunctionType.Sigmoid,
                bias=b_bcast[:, 0:1],
            )

            # hoisted x1 loads on the SWDGE (gpsimd) queue. Group 0 is
            # split into two 2-tile DMAs: shorter Q7 descriptor
            # emission, so the first transfer (and the whole B_i chain
            # behind it) starts ~1us earlier. Remaining groups are 4
            # tiles (656 KB) each.
            xt0a = xpool.tile([P, 2, F], f32, name="xt0a", tag="xt0a", bufs=1)
            nc.gpsimd.dma_start(
                out=xt0a, in_=x_d[0 : 2 * P, :].rearrange("(t p) f -> p t f", p=P)
            )
            xt0b = xpool.tile([P, 2, F], f32, name="xt0b", tag="xt0b", bufs=1)
            nc.gpsimd.dma_start(
                out=xt0b,
                in_=x_d[2 * P : 4 * P, :].rearrange("(t p) f -> p t f", p=P),
            )
            xts = [None]
            for g in range(1, COL_TILES // LOAD_GROUP):
                xt = xpool.tile(
                    [P, LOAD_GROUP, F], f32, name=f"xt{g}", tag=f"xt{g}", bufs=1
                )
                src = x_d[
                    g * LOAD_GROUP * P : (g + 1) * LOAD_GROUP * P, :
                ].rearrange("(t p) f -> p t f", p=P)
                nc.gpsimd.dma_start(out=xt, in_=src)
                xts.append(xt)

            def col_tile(j):
                if j < 2:
                    return xt0a[:, j, :]
                if j < 4:
                    return xt0b[:, j - 2, :]
                return xts[j // LOAD_GROUP][:, j % LOAD_GROUP, :]

            # B_i = p_i + conv_b, broadcast across partitions (once).
            # Own rows are column tiles 0..7 thanks to the roll. One
            # batched mul+reduce per 4-tile load group.
            w_b_g = bass.AP(
                tensor=w_rep.tensor,
                offset=w_b_rep.offset,
                ap=[w_rep.ap[0], [0, LOAD_GROUP], [1, F]],
            )
            w_a_g = bass.AP(
                tensor=w_rep.tensor,
                offset=w_a_rep.offset,
                ap=[w_rep.ap[0], [0, LOAD_GROUP], [1, F]],
            )
            w_b_g2 = bass.AP(
                tensor=w_rep.tensor,
                offset=w_b_rep.offset,
                ap=[w_rep.ap[0], [0, 2], [1, F]],
            )
            bi_sb = singles.tile([P, ROWS_PER_CORE], f32)
            HG = ROW_TILES // 2  # 4 row tiles per half-chain
            for h in range(2):
                # independent half-chain: gated only by its own 4-tile
                # dot group, so the first activations start early
                pib = small.tile([P, HG], f32, name=f"pib{h}", tag="pib", bufs=2)
                if h == 0:
                    # two 2-tile pairs matching the split group-0 loads
                    for q, xh in enumerate((xt0a, xt0b)):
                        scr = small.tile(
                            [P, 2, F], f32, name=f"scri0{q}", tag="scr2", bufs=2
                        )
                        nc.vector.tensor_mul(out=scr, in0=xh, in1=w_b_g2)
                        nc.vector.reduce_sum(
                            out=pib[:, q * 2 : (q + 1) * 2],
                            in_=scr,
                            axis=mybir.AxisListType.X,
                        )
                else:
                    scr = small.tile(
                        [P, HG, F], f32, name=f"scri{h}", tag="scrg", bufs=2
                    )
                    nc.vector.tensor_mul(out=scr, in0=xts[h], in1=w_b_g)
                    nc.vector.reduce_sum(
                        out=pib, in_=scr, axis=mybir.AxisListType.X
                    )

                piT_ps = pst.tile([HG, P], f32, name=f"piTps{h}", tag="piTps", bufs=2)
                nc.tensor.transpose(piT_ps, pib, ident)
                piT = small.tile([HG, P], f32, name=f"piT{h}", tag="piT", bufs=2)
                nc.vector.tensor_copy(out=piT, in_=piT_ps)

                rhs = small.tile(
                    [HG, HG, P], f32, name=f"rhs{h}", tag="rhs", bufs=2
                )
                piT_b = bass.AP(
                    tensor=piT.tensor,
                    offset=piT.offset,
                    ap=[piT.ap[0], [0, HG], piT.ap[1]],
                )
                identh_b = bass.AP(
                    tensor=ident.tensor,
                    offset=ident.offset,
                    ap=[[ident.ap[0][0], HG], [ident.ap[1][0], HG], [0, P]],
                )
                nc.vector.tensor_tensor(
                    out=rhs, in0=piT_b, in1=identh_b, op=mybir.AluOpType.mult
                )

                bch = psum.tile([P, BANK], f32, name=f"bc{h}", tag=f"bc{h}", bufs=1)
                nc.tensor.matmul(
                    bch,
                    ones_k[0:HG, :],
                    rhs,
                    start=True,
                    stop=True,
                )
                # PSUM -> SBUF copy with the conv_b add folded in
                nc.vector.tensor_scalar_add(
                    out=bi_sb[:, h * BANK : (h + 1) * BANK],
                    in0=bch,
                    scalar1=b_bcast[:, 0:1],
                )

            # main loop: one dot, one activation, one fully-contiguous
            # 512 KB store per j tile (fine granularity keeps the DVE
            # ahead of the ScalarEngine's activation stream)
            for j in range(COL_TILES):
                scr = small.tile([P, F], f32, name=f"scrj{j}", tag="scr", bufs=4)
                pjv = small.tile([P, 1], f32, name=f"pjv{j}", tag="pjv", bufs=4)
                nc.vector.tensor_mul(out=scr, in0=col_tile(j), in1=w_a_rep)
                nc.vector.reduce_sum(
                    out=pjv, in_=scr, axis=mybir.AxisListType.X
                )
                ot = outp.tile(
                    [P, ROWS_PER_CORE], f32, name=f"ot{j}", tag="ot", bufs=8
                )
                nc.scalar.activation(
                    out=ot,
                    in_=bi_sb,
                    func=mybir.ActivationFunctionType.Sigmoid,
                    bias=pjv,
                    scale=1.0,
                )
                nc.sync.dma_start(
                    out=o_d[j * P : (j + 1) * P, :],
                    in_=ot,
                )

    if fixup:
        _split_multiwait_instructions(nc)
    return nc


_NC = None


def _get_program():
    global _NC
    if _NC is None:
        _NC = _build_program()
    return _NC


def _run_spmd(x1, conv_w, conv_b, trace=False, **run_kwargs):
    x1 = np.ascontiguousarray(x1, dtype=np.float32)
    conv_w = np.ascontiguousarray(conv_w, dtype=np.float32)
    conv_b = np.ascontiguousarray(conv_b, dtype=np.float32)
    ident = np.eye(P, dtype=np.float32)
    ones = np.ones((ROW_TILES, P), dtype=np.float32)

    nc = _get_program()
    in_maps = []
    for k in range(N_CORES):
        b, m = divmod(k, BLOCKS_PER_BATCH)
        x1r = np.ascontiguousarray(np.roll(x1[b], -ROWS_PER_CORE * m, axis=0))
        in_maps.append(
            {
                "x1r": x1r,
                "conv_w": conv_w,
                "conv_b": conv_b,
                "ident": ident,
                "ones": ones,
            }
        )

    res = bass_utils.run_bass_kernel_spmd(
        nc, in_maps, core_ids=list(range(N_CORES)), trace=trace, **run_kwargs
    )

    out = np.empty((B, N, N), dtype=np.float32)
    for k in range(N_CORES):
        b, m = divmod(k, BLOCKS_PER_BATCH)
        blk = res.results[k]["out"]  # [N(j, rolled), ROWS_PER_CORE(i)]
        out[b, m * ROWS_PER_CORE : (m + 1) * ROWS_PER_CORE, :] = np.roll(
            blk, ROWS_PER_CORE * m, axis=0
        ).T
    return out, res


def kernel(x1, conv_w, conv_b):
    return _run_spmd(x1, conv_w, conv_b)[0]



# revision 2
# speedup vs baseline: 1.1239x; 1.1239x over previous
"""Trainium2 Bass kernel for nn_Concat_Model_89343909692135.

Computes out[b,i,j] = sigmoid(w_b.x1[b,i] + w_a.x1[b,j] + bias) for
B=2, N=4096, F=320, distributed over 8 NeuronCores.

Sharding: core k handles batch b = k//4, row block m = k%4 (1024 rows).
Each core receives x1[b] TRANSPOSED to [F, N] in fp16, with the j axis
rolled so its own 1024 rows come first, and writes its output block
TRANSPOSED in bf16: out_t[j, i] with j = all 4096 (rolled) column
nodes on the partition axis and i = the core's 1024 own rows on the
free axis. The host un-rolls, transposes and upcasts to fp32.

Key layout/dtype choices vs the fp32 row-major baseline:
  - fp16 x1 halves the input DMA traffic (2.6 MB/core); the dot
    products accumulate in fp32 PSUM so precision stays ~1e-3.
  - bf16 output halves the store traffic (8.4 MB/core); sigmoid
    outputs are in [0,1] so bf16's 2^-9 relative step is ~2e-3 error,
    well inside the 2e-2 gate.
  - the transposed x1 layout moves both dot-product reductions onto
    the Tensor engine (nearly free) instead of the DVE:
      p_i row (broadcast over partitions): lhsT = w_b replicated
        [K=f-chunk, 128], rhs = x1T own columns -> PSUM [128, 1024].
      p_j column: lhsT = x1T j-tile [K, 128], rhs = w_a [K, 1]
        -> PSUM [128, 1] per tile.
    The conv bias is folded into the p_i matmul via a ones row
    appended to the last f-chunk (row 64) with w_b[64] = bias.
  - per j-tile: one Sigmoid activation out = sigmoid(bi + bias=p_j)
    on ScalarE reading bi straight from PSUM, bf16 out to SBUF, and
    one 512 KB paired store (2 j-tiles) on the sync HWDGE queue.
"""

import numpy as np

import concourse.bass as bass
import concourse.mybir as mybir
import concourse.tile as tile
from concourse import bass_utils

B = 2
N = 4096
F = 320
P = 128
N_CORES = 8
BLOCKS_PER_BATCH = N_CORES // B  # 4
ROWS_PER_CORE = N // BLOCKS_PER_BATCH  # 1024
COL_TILES = N // P  # 32
FCH = (128, 128, 65)  # f-chunks; chunk 2 has the ones row appended
BANK = 512  # fp32 elements per PSUM bank
N_DUMMY = 12  # PE warm-up matmuls


def _split_multiwait_instructions(nc):
    # The walrus build here only accepts one sem-wait per instruction.
    # Hoist extra waits onto preceding NoOps on the same engine queue;
    # in-order execution per engine makes this equivalent.
    seen_dma = False
    for fn in nc.m.functions:
        for bb in fn.blocks:
            new_list = []
            for ins in bb.instructions:
                # strip the all-engine ENTRY barrier (drain + EVSEM
                # butterfly before any real work): engines enter with
                # clean state (the exit sequence cleared sems) and all
                # real cross-engine deps are explicit Tile semaphores
                nm = type(ins).__name__
                if nm == "InstDMACopy":
                    seen_dma = True
                if not seen_dma and nm in ("InstDrain", "InstEventSemaphore"):
                    continue
                # drop the framework's unused const-tile memsets (the
                # verifier flags them as having no reader); they sit at
                # the head of the Pool queue and delay the first x1
                # load emission
                if (
                    type(ins).__name__ == "InstMemset"
                    and ins.outs
                    and getattr(ins.outs[0], "memref", "")
                    in (
                        "const-float32-0.0",
                        "const-float32-1.0",
                        "const-bfloat16-1.0",
                        "const-uint8-127",
                    )
                ):
                    continue
                si = getattr(ins, "sync_info", None)
                if si is not None and si.on_wait and len(si.on_wait) > 1:
                    waits = list(si.on_wait)
                    for i, w in enumerate(waits[:-1]):
                        nop = mybir.InstNoOp(
                            name=f"{ins.name}-w{i}",
                            ins=[],
                            outs=[],
                            engine=ins.engine,
                            sync_info=type(si)(on_wait=[w], on_update=[]),
                        )
                        new_list.append(nop)
                    si.on_wait = waits[-1:]
                new_list.append(ins)
            bb.instructions[:] = new_list


def _build_program(fixup=True):
    nc = bass.Bass("TRN2", debug=False, target_bir_lowering=False)
    f32 = mybir.dt.float32
    f16 = mybir.dt.float16
    bf16 = mybir.dt.bfloat16

    xt_d = [
        nc.dram_tensor(f"x1t{c}", [FCH[c], N], f16, kind="ExternalInput").ap()
        for c in range(3)
    ]
    # per f-chunk [pc, 2]: col 0 = w_a, col 1 = w_b (+ bias in chunk 2 row 64)
    w_d = [
        nc.dram_tensor(f"wab{c}", [FCH[c], 2], f16, kind="ExternalInput").ap()
        for c in range(3)
    ]
    o_d = nc.dram_tensor("out", [N, ROWS_PER_CORE], bf16, kind="ExternalOutput").ap()

    with tile.TileContext(nc) as tc:
        with (
            tc.tile_pool(name="singles", bufs=1) as singles,
            tc.tile_pool(name="xpool", bufs=1) as xpool,
            tc.tile_pool(name="outp", bufs=4) as outp,
            tc.tile_pool(name="psbi", bufs=1, space="PSUM") as psbi,
            tc.tile_pool(name="pspj", bufs=1, space="PSUM") as pspj,
        ):
            # --- scratch + PE warm-up fodder (memsets, DVE queue) ---
            warm_in = singles.tile([P, 1], f32)
            dml = singles.tile([P, P], f16)
            dmr = singles.tile([P, 256], f16)
            nc.vector.memset(warm_in, 0.0)
            nc.vector.memset(dml, 0.0)
            nc.vector.memset(dmr, 0.0)

            # --- tiny weight loads (sync HWDGE queue) ---
            w_sb = []
            for c in range(3):
                wt = singles.tile([FCH[c], 2], f16, name=f"w{c}")
                nc.sync.dma_start(out=wt, in_=w_d[c])
                w_sb.append(wt)

            # --- x1T loads: own columns first, then the rest, SWDGE ---
            xt = [
                xpool.tile([FCH[c], N], f16, name=f"xt{c}", tag=f"xt{c}", bufs=1)
                for c in range(3)
            ]
            for c in range(3):
                nc.gpsimd.dma_start(
                    out=xt[c][:, 0:ROWS_PER_CORE],
                    in_=xt_d[c][:, 0:ROWS_PER_CORE],
                )
            for lo, hi in ((1024, 2560), (2560, 4096)):
                for c in range(3):
                    nc.gpsimd.dma_start(
                        out=xt[c][:, lo:hi], in_=xt_d[c][:, lo:hi]
                    )

            # --- warm-up: sigmoid ACT-table load + PE pipeline ramp ---
            warm = singles.tile([P, 1], f32)
            nc.scalar.activation(
                out=warm,
                in_=warm_in,
                func=mybir.ActivationFunctionType.Sigmoid,
                bias=0.0,
            )
            pj_bank = pspj.tile([P, BANK], f32)
            for d in range(N_DUMMY):
                nc.tensor.matmul(
                    pj_bank[:, 256:512],
                    dml,
                    dmr,
                    start=True,
                    stop=True,
                )

            # --- w_b broadcast tiles [pc, 128] for the p_i row matmul ---
            wb_bc = []
            for c in range(3):
                wbt = singles.tile([FCH[c], P], f16, name=f"wbb{c}")
                src = bass.AP(
                    tensor=w_sb[c].tensor,
                    offset=w_sb[c][:, 1:2].offset,
                    ap=[w_sb[c].ap[0], [0, P]],
                )
                nc.vector.tensor_copy(out=wbt, in_=src)
                wb_bc.append(wbt)

            # --- bi = p_i + bias, broadcast over partitions, in PSUM ---
            bi_ps = psbi.tile([P, ROWS_PER_CORE], f32)
            for h in range(2):
                for c in range(3):
                    nc.tensor.matmul(
                        bi_ps[:, h * BANK : (h + 1) * BANK],
                        wb_bc[c],
                        xt[c][:, h * BANK : (h + 1) * BANK],
                        start=(c == 0),
                        stop=(c == 2),
                    )

            # --- p_j columns: 3 accumulating matmuls per j-tile ---
            for jt in range(COL_TILES):
                for c in range(3):
                    nc.tensor.matmul(
                        pj_bank[:, jt : jt + 1],
                        xt[c][:, jt * P : (jt + 1) * P],
                        w_sb[c][:, 0:1],
                        start=(c == 0),
                        stop=(c == 2),
                    )

            # p_j PSUM -> SBUF in 8-column chunks so early sigmoids
            # don't wait on late columns
            pj_sb = singles.tile([P, COL_TILES], f32)
            for g in range(4):
                nc.vector.tensor_copy(
                    out=pj_sb[:, g * 8 : (g + 1) * 8],
                    in_=pj_bank[:, g * 8 : (g + 1) * 8],
                )

            # --- main loop: sigmoid + paired stores ---
            for pr in range(COL_TILES // 2):
                ot = outp.tile(
                    [P, 2, ROWS_PER_CORE], bf16, name=f"ot{pr}", tag="ot", bufs=4
                )
                for t in range(2):
                    jt = pr * 2 + t
                    nc.scalar.activation(
                        out=ot[:, t, :],
                        in_=bi_ps,
                        func=mybir.ActivationFunctionType.Sigmoid,
                        bias=pj_sb[:, jt : jt + 1],
                        scale=1.0,
                    )
                nc.sync.dma_start(
                    out=o_d[pr * 2 * P : (pr + 1) * 2 * P, :].rearrange(
                        "(t p) i -> p t i", p=P
                    ),
                    in_=ot,
                )

    if fixup:
        _split_multiwait_instructions(nc)
    return nc


_NC = None


def _get_program():
    global _NC
    if _NC is None:
        _NC = _build_program()
    return _NC


def _prep_inputs(x1, conv_w, conv_b):
    x1 = np.ascontiguousarray(x1, dtype=np.float32)
    conv_w = np.asarray(conv_w, dtype=np.float32)
    conv_b = np.asarray(conv_b, dtype=np.float32)
    f = F
    w_a = conv_w[:f]
    w_b = conv_w[f:]
    # [321, 2] fp16: col 0 = w_a (+0 pad row), col 1 = w_b (+bias row)
    wab = np.zeros((F + 1, 2), dtype=np.float16)
    wab[:F, 0] = w_a.astype(np.float16)
    wab[:F, 1] = w_b.astype(np.float16)
    wab[F, 1] = np.float16(conv_b[0])
    w_chunks = [wab[0:128], wab[128:256], wab[256 : F + 1]]

    in_maps = []
    for k in range(N_CORES):
        b, m = divmod(k, BLOCKS_PER_BATCH)
        x1t = np.roll(x1[b], -ROWS_PER_CORE * m, axis=0).T.astype(np.float16)
        x1t_aug = np.empty((F + 1, N), dtype=np.float16)
        x1t_aug[:F] = x1t
        x1t_aug[F] = np.float16(1.0)
        in_maps.append(
            {
                "x1t0": np.ascontiguousarray(x1t_aug[0:128]),
                "x1t1": np.ascontiguousarray(x1t_aug[128:256]),
                "x1t2": np.ascontiguousarray(x1t_aug[256 : F + 1]),
                "wab0": w_chunks[0],
                "wab1": w_chunks[1],
                "wab2": w_chunks[2],
            }
        )
    return in_maps


def _run_spmd(x1, conv_w, conv_b, trace=False, **run_kwargs):
    in_maps = _prep_inputs(x1, conv_w, conv_b)
    nc = _get_program()
    res = bass_utils.run_bass_kernel_spmd(
        nc, in_maps, core_ids=list(range(N_CORES)), trace=trace, **run_kwargs
    )

    out = np.empty((B, N, N), dtype=np.float32)
    for k in range(N_CORES):
        b, m = divmod(k, BLOCKS_PER_BATCH)
        blk = np.asarray(res.results[k]["out"]).astype(np.float32)
        out[b, m * ROWS_PER_CORE : (m + 1) * ROWS_PER_CORE, :] = np.roll(
            blk, ROWS_PER_CORE * m, axis=0
        ).T
    return out, res


def kernel(x1, conv_w, conv_b):
    return _run_spmd(x1, conv_w, conv_b)[0]


# revision 50
# speedup vs baseline: 1.7766x; 1.5808x over previous
"""Trainium2 Bass kernel for nn_Concat_Model_89343909692135.

Computes out[b,i,j] = sigmoid(w_b.x1[b,i] + w_a.x1[b,j] + bias) for
B=2, N=4096, F=320, distributed over 8 NeuronCores.

Sharding: core k handles batch b = k//4, row block m = k%4 (1024 rows).
Each core receives x1[b] TRANSPOSED to [F, N] in fp16, with the j axis
rolled so its own 1024 rows come first, and writes its output block
TRANSPOSED in bf16: out_t[j, i] with j = all 4096 (rolled) column
nodes on the partition axis and i = the core's 1024 own rows on the
free axis. The host un-rolls, transposes and upcasts to fp32.

Layout / dtype / engine-split choices (cost-model driven: all DMA
transfers serialize on one ~360 GB/s resource, so total bytes set the
floor; ScalarE runs ~1.04us per 128x1024 sigmoid tile):
  - fp16 x1 halves input DMA traffic (2.6 MB/core); dots accumulate in
    fp32 PSUM so the error stays ~1e-3 (gate is 2e-2). w_a/w_b (+ the
    conv bias via a ones row appended to the last f-chunk) are packed
    as the first two columns of each x1T chunk so no separate weight
    DMAs sit ahead of the column loads.
  - bf16 output halves store traffic (8.4 MB/core); sigmoid outputs
    are in [0,1] so bf16 costs ~2e-3 relative.
  - x1 loads ride the sync-queue HWDGE (cheap descriptor gen), own
    columns first, then the late offload columns, then the mid ones;
    ACT-path stores ride the Pool SWDGE queue and offload stores ride
    sync HWDGE so no stream queues behind another. Early tiles store
    singly to fill the DMA window right after the loads drain.
  - the transposed x1 layout puts both dot reductions on the Tensor
    engine: p_i+bias rows via w_b-replicated lhsT, p_j columns via x1T
    j-tile lhsT with the w_a column as rhs. p_j lives in three PSUM
    tiles split at load-piece boundaries so early sigmoid biases and
    the v transposes don't wait on late column loads (dependency
    tracking is per-tile). bi is copied PSUM->SBUF once (ACT reads
    SBUF ~0.2us/op faster than PSUM) and its PSUM banks are recycled
    through the q-tile ring.
  - j-tiles 0..3 and 8..17 go through ScalarE sigmoid(bi + p_j[jt]).
  - j-tiles 4..7 and 18..31 are offloaded: with u = e^-(p_i+b) and
    v = e^-p_j (tiny ScalarE exps, fp16, fed by PE transposes of the
    p_j blocks), PE accumulates q = 1 + v_j*u_i per tile (rank-1
    matmul + ones matmul into PSUM) and the DVE finishes with a single
    reciprocal: out = 1/q = sigmoid(raw). The early group (4..7) runs
    while the sigmoid stream ramps; the split keeps every engine under
    the DMA roofline (~31us busy, ~36.9us end to end vs 65.6us for
    the fp32 row-major baseline).
"""

import numpy as np

import concourse.bass as bass
import concourse.mybir as mybir
import concourse.tile as tile
from concourse import bass_utils

B = 2
N = 4096
F = 320
P = 128
N_CORES = 8
BLOCKS_PER_BATCH = N_CORES // B  # 4
ROWS_PER_CORE = N // BLOCKS_PER_BATCH  # 1024
COL_TILES = N // P  # 32
FCH = (128, 128, 65)  # f-chunks; chunk 2 has the ones row appended
BANK = 512  # fp32 elements per PSUM bank
OFF0 = 18  # first offloaded j-tile (tiles OFF0..31 take the exp path)
SPLIT = OFF0 * P  # x1T load-piece boundary (offload columns load first)
WOFF = 2  # w_a/w_b packed as the first two columns of each x1t chunk


def _split_multiwait_instructions(nc):
    # The walrus build here only accepts one sem-wait per instruction.
    # Hoist extra waits onto preceding NoOps on the same engine queue;
    # in-order execution per engine makes this equivalent.
    seen_dma = False
    for fn in nc.m.functions:
        for bb in fn.blocks:
            new_list = []
            for ins in bb.instructions:
                # strip the all-engine ENTRY barrier (drain + EVSEM
                # butterfly before any real work): engines enter with
                # clean state (the exit sequence cleared sems) and all
                # real cross-engine deps are explicit Tile semaphores
                nm = type(ins).__name__
                if nm == "InstDMACopy":
                    seen_dma = True
                if not seen_dma and nm in ("InstDrain", "InstEventSemaphore"):
                    continue
                # drop the framework's unused const-tile memsets (the
                # verifier flags them as having no reader); they sit at
                # the head of the Pool queue and delay the first store
                # emission
                if (
                    type(ins).__name__ == "InstMemset"
                    and ins.outs
                    and getattr(ins.outs[0], "memref", "")
                    in (
                        "const-float32-0.0",
                        "const-float32-1.0",
                        "const-bfloat16-1.0",
                        "const-uint8-127",
                    )
                ):
                    continue
                si = getattr(ins, "sync_info", None)
                if si is not None and si.on_wait and len(si.on_wait) > 1:
                    waits = list(si.on_wait)
                    for i, w in enumerate(waits[:-1]):
                        nop = mybir.InstNoOp(
                            name=f"{ins.name}-w{i}",
                            ins=[],
                            outs=[],
                            engine=ins.engine,
                            sync_info=type(si)(on_wait=[w], on_update=[]),
                        )
                        new_list.append(nop)
                    si.on_wait = waits[-1:]
                new_list.append(ins)
            bb.instructions[:] = new_list


def _build_program(fixup=True):
    nc = bass.Bass("TRN2", debug=False, target_bir_lowering=False)
    f32 = mybir.dt.float32
    f16 = mybir.dt.float16
    bf16 = mybir.dt.bfloat16

    # per f-chunk [pc, 2 + N]: cols 0:2 = (w_a, w_b) for that chunk
    # (chunk 2 row 64 = bias row), cols 2: = x1T columns (rolled)
    xt_d = [
        nc.dram_tensor(f"x1t{c}", [FCH[c], WOFF + N], f16, kind="ExternalInput").ap()
        for c in range(3)
    ]
    id_d = nc.dram_tensor("ident", [P, P], f32, kind="ExternalInput").ap()
    o_d = nc.dram_tensor("out", [N, ROWS_PER_CORE], bf16, kind="ExternalOutput").ap()

    n_act = OFF0  # j-tiles on the ScalarE sigmoid path
    n_off = COL_TILES - OFF0  # j-tiles on the PE+DVE exp path

    with tile.TileContext(nc) as tc:
        with (
            tc.tile_pool(name="singles", bufs=1) as singles,
            tc.tile_pool(name="xpool", bufs=1) as xpool,
            tc.tile_pool(name="outp", bufs=4) as outp,
            tc.tile_pool(name="outo", bufs=3) as outo,
            tc.tile_pool(name="psbi", bufs=1, space="PSUM") as psbi,
            tc.tile_pool(name="pspj", bufs=1, space="PSUM") as pspj,
            tc.tile_pool(name="psq", bufs=2, space="PSUM") as psq,
        ):
            # --- constants (DVE memsets, ready ~instantly) ---
            warm_in = singles.tile([P, 1], f32)
            zbias = singles.tile([P, 1], f32)  # AP bias: imm bias is
            # mis-encoded on the walrus functional model (adds junk)
            ones1 = singles.tile([1, P], f16)  # K=1 lhsT of the +1 matmul
            onesi = singles.tile([1, ROWS_PER_CORE], f16)  # its rhs
            nc.vector.memset(warm_in, 0.0)
            nc.vector.memset(zbias, 0.0)
            nc.vector.memset(ones1, 1.0)
            nc.vector.memset(onesi, 1.0)

            # --- x1T loads (sync HWDGE): own columns (+ the packed w
            # cols) first, then [SPLIT:4096] (the late offload tiles'
            # p_j columns gate the DVE stream), then the mid columns
            # which only feed sigmoid biases needed later. The identity
            # (needed ~6us for the p_j transposes) rides the otherwise
            # idle Pool SWDGE queue. ---
            xt = [
                xpool.tile(
                    [FCH[c], WOFF + N], f16, name=f"xt{c}", tag=f"xt{c}",
                    bufs=1,
                )
                for c in range(3)
            ]
            for lo, hi in (
                (0, WOFF + ROWS_PER_CORE),
                (WOFF + SPLIT, WOFF + N),
                (WOFF + 1024, WOFF + SPLIT),
            ):
                for c in range(3):
                    nc.sync.dma_start(
                        out=xt[c][:, lo:hi], in_=xt_d[c][:, lo:hi]
                    )
            ident = singles.tile([P, P], f32)
            nc.gpsimd.dma_start(out=ident, in_=id_d)

            # warm-up: trigger the sigmoid ACT-table load early (real-HW
            # cost; free in the cost model)
            warm = singles.tile([P, 1], f32)
            nc.scalar.activation(
                out=warm,
                in_=warm_in,
                func=mybir.ActivationFunctionType.Sigmoid,
                bias=zbias[:, 0:1],
            )

            # --- w_b broadcast tiles [pc, 128] for the p_i row matmul ---
            wb_bc = []
            for c in range(3):
                wbt = singles.tile([FCH[c], P], f16, name=f"wbb{c}")
                wcol = xt[c][:, 1:2]
                bcast = bass.AP(
                    tensor=wcol.tensor,
                    offset=wcol.offset,
                    ap=[wcol.ap[0], [0, P]],
                )
                nc.vector.tensor_copy(out=wbt, in_=bcast)
                wb_bc.append(wbt)

            # --- bi = p_i + bias (broadcast over partitions): PE matmul
            # into PSUM per 512-half, then DVE copy to SBUF (ACT reads
            # SBUF ~0.2us/op faster than PSUM) ---
            # bi shares the q-tile PSUM ring (tag "q"): its banks are
            # recycled by the third offload tile, after the copies below
            bi_ps = psq.tile([P, ROWS_PER_CORE], f32, name="bi", tag="q", bufs=2)
            bi_sb = singles.tile([P, ROWS_PER_CORE], f32)
            for h in range(2):
                for c in range(3):
                    nc.tensor.matmul(
                        bi_ps[:, h * BANK : (h + 1) * BANK],
                        wb_bc[c],
                        xt[c][:, WOFF + h * BANK : WOFF + (h + 1) * BANK],
                        start=(c == 0),
                        stop=(c == 2),
                    )

            # --- p_j columns; three PSUM tiles split at load-piece
            # boundaries so early biases / the v transpose don't wait on
            # unrelated column loads (dependency tracking is per-tile) ---
            pjA = pspj.tile([P, BANK], f32, name="pjA", tag="pjA", bufs=1)
            pjB = pspj.tile([P, BANK], f32, name="pjB", tag="pjB", bufs=1)
            pjC = pspj.tile([P, BANK], f32, name="pjC", tag="pjC", bufs=1)

            def pj_mms(jt_lo, jt_hi, bank, col0):
                for jt in range(jt_lo, jt_hi):
                    for c in range(3):
                        nc.tensor.matmul(
                            bank[:, jt - col0 : jt - col0 + 1],
                            xt[c][:, WOFF + jt * P : WOFF + (jt + 1) * P],
                            xt[c][:, 0:1],
                            start=(c == 0),
                            stop=(c == 2),
                        )

            pj_mms(0, 8, pjA, 0)

            # DVE: bi PSUM -> SBUF per half (after the B_row matmuls)
            for h in range(2):
                nc.vector.tensor_copy(
                    out=bi_sb[:, h * BANK : (h + 1) * BANK],
                    in_=bi_ps[:, h * BANK : (h + 1) * BANK],
                )

            pjA_sb = singles.tile([P, 8], f32)
            with tc.high_priority():
                nc.vector.tensor_copy(out=pjA_sb, in_=pjA[:, 0:8])

            # --- exp-path ingredients. u = e^-(p_i+b) row (shared);
            # v = e^-p_j rows per offloaded tile, produced by a PSUM
            # transpose of the p_j block + one tiny Exp, then flattened
            # onto partition 0 with a small SBUF->SBUF DMA (PE lhsT
            # operands must sit at base partition 0, so the per-tile
            # rows are sliced along the free dim instead) ---
            u_row = singles.tile([1, ROWS_PER_CORE], f16)
            with tc.high_priority():
                nc.scalar.activation(
                    out=u_row,
                    in_=bi_sb[0:1, :],
                    func=mybir.ActivationFunctionType.Exp,
                    scale=-1.0,
                    bias=zbias[0:1, 0:1],
                )

            def make_v(pj_src, n, psum_col, eng=None):
                pjT = bass.AP(
                    tensor=pjA.tensor,
                    offset=pjA[0:n, psum_col : psum_col + P].offset,
                    ap=[[pjA.ap[0][0], n], [1, P]],
                )
                vr = singles.tile([n, P], f16, name=f"vr{psum_col}")
                vf = singles.tile([1, n * P], f16, name=f"vf{psum_col}")
                with tc.high_priority():
                    nc.tensor.transpose(pjT, pj_src, ident)
                    nc.scalar.activation(
                        out=vr,
                        in_=pjT,
                        func=mybir.ActivationFunctionType.Exp,
                        scale=-1.0,
                        bias=zbias[0:n, 0:1],
                    )
                    vf_out = bass.AP(
                        tensor=vf.tensor,
                        offset=vf.offset,
                        ap=[[vf.ap[0][0], 1], [P, n], [1, P]],
                    )
                    (eng or nc.scalar).dma_start(out=vf_out, in_=vr)
                return vf

            # early offload group: tiles 4..7 factor through pjA, whose
            # columns are ready ~6us in -- their PE+DVE work and stores
            # fill the DMA window while the sigmoid stream ramps
            v_early = make_v(pjA_sb[:, 4:8], 4, 2 * P, eng=nc.sync)

            # --- output tiles + stores. Early tiles store singly (fills
            # the DMA window right after the loads drain); later tiles
            # store in pairs. ACT-path stores ride the Pool SWDGE queue;
            # offload stores ride sync HWDGE (free after the loads) so
            # neither stream queues behind the other ---
            off_tiles = set(range(4, 8)) | set(range(OFF0, COL_TILES))
            single_set = {0, 1, 2, 3, 4, 5, OFF0, OFF0 + 1}
            ot_sing = {
                j: (outo if j in off_tiles else outp).tile(
                    [P, ROWS_PER_CORE], bf16, name=f"os{j}",
                    tag="oto" if j in off_tiles else "ot",
                    bufs=3 if j in off_tiles else 4,
                )
                for j in sorted(single_set)
            }
            ot_pair = {}
            for pr in range(3, COL_TILES // 2):
                if pr * 2 in single_set:
                    continue
                pool, tag = (
                    (outo, "otq") if pr * 2 in off_tiles else (outp, "otp")
                )
                ot_pair[pr] = pool.tile(
                    [P, 2, ROWS_PER_CORE], bf16, name=f"ot{pr}", tag=tag,
                    bufs=3,
                )

            def out_ap(jt):
                if jt in single_set:
                    return ot_sing[jt][:, :]
                return ot_pair[jt // 2][:, jt % 2, :]

            def emit_store(jt):
                eng = nc.sync if jt in off_tiles else nc.gpsimd
                if jt in single_set:
                    eng.dma_start(
                        out=o_d[jt * P : (jt + 1) * P, :], in_=ot_sing[jt]
                    )
                elif jt % 2 == 1:
                    pr = jt // 2
                    eng.dma_start(
                        out=o_d[pr * 2 * P : (pr + 1) * 2 * P, :].rearrange(
                            "(t p) i -> p t i", p=P
                        ),
                        in_=ot_pair[pr],
                    )

            # --- offloaded tiles: q = 1 + v_j (x) u_i on PE (rank-1
            # matmul + ones matmul into PSUM), then one DVE reciprocal
            # -> sigmoid ---
            def emit_offload(tiles, vf, base):
                with nc.allow_low_precision(reason="bf16 sigmoid out"), \
                        tc.high_priority():
                    for jt in tiles:
                        q = psq.tile(
                            [P, ROWS_PER_CORE], f32, name=f"q{jt}", tag="q",
                            bufs=2,
                        )
                        for h in range(2):
                            hs = slice(h * BANK, (h + 1) * BANK)
                            nc.tensor.matmul(
                                q[:, hs],
                                vf[:, (jt - base) * P : (jt - base + 1) * P],
                                u_row[:, hs],
                                start=True,
                                stop=False,
                            )
                            nc.tensor.matmul(
                                q[:, hs],
                                ones1,
                                onesi[:, hs],
                                start=False,
                                stop=True,
                            )
                        nc.vector.reciprocal(out=out_ap(jt), in_=q)
                        emit_store(jt)

            emit_offload(range(4, 8), v_early, 4)

            pj_mms(OFF0, 32, pjC, OFF0)
            pjC_sb = singles.tile([P, n_off], f32)
            with tc.high_priority():
                nc.vector.tensor_copy(out=pjC_sb, in_=pjC[:, 0:n_off])
            v_late = make_v(pjC_sb, n_off, P)

            pjB_sb = singles.tile([P, OFF0 - 8], f32)

            def emit_pjB():
                pj_mms(8, OFF0, pjB, 8)
                nc.vector.tensor_copy(out=pjB_sb, in_=pjB[:, 0 : OFF0 - 8])

            for jt in list(range(0, 4)) + list(range(8, OFF0)):
                bias = (
                    pjA_sb[:, jt : jt + 1]
                    if jt < 8
                    else pjB_sb[:, jt - 8 : jt - 7]
                )
                nc.scalar.activation(
                    out=out_ap(jt),
                    in_=bi_sb,
                    func=mybir.ActivationFunctionType.Sigmoid,
                    bias=bias,
                    scale=1.0,
                )
                emit_store(jt)
                if jt == 0:
                    emit_pjB()

            emit_offload(range(OFF0, COL_TILES), v_late, OFF0)

    if fixup:
        _split_multiwait_instructions(nc)
    return nc


_NC = None


def _get_program():
    global _NC
    if _NC is None:
        _NC = _build_program()
    return _NC


def _prep_inputs(x1, conv_w, conv_b):
    x1 = np.ascontiguousarray(x1, dtype=np.float32)
    conv_w = np.asarray(conv_w, dtype=np.float32)
    conv_b = np.asarray(conv_b, dtype=np.float32)
    f = F
    w_a = conv_w[:f]
    w_b = conv_w[f:]
    # [321, 2] fp16: col 0 = w_a (+0 pad row), col 1 = w_b (+bias row),
    # packed as the first two columns of the augmented x1T rows
    wab = np.zeros((F + 1, 2), dtype=np.float16)
    wab[:F, 0] = w_a.astype(np.float16)
    wab[:F, 1] = w_b.astype(np.float16)
    wab[F, 1] = np.float16(conv_b[0])
    ident = np.eye(P, dtype=np.float32)

    in_maps = []
    for k in range(N_CORES):
        b, m = divmod(k, BLOCKS_PER_BATCH)
        x1t = np.roll(x1[b], -ROWS_PER_CORE * m, axis=0).T.astype(np.float16)
        x1t_aug = np.empty((F + 1, WOFF + N), dtype=np.float16)
        x1t_aug[:, :WOFF] = wab
        x1t_aug[:F, WOFF:] = x1t
        x1t_aug[F, WOFF:] = np.float16(1.0)
        in_maps.append(
            {
                "x1t0": np.ascontiguousarray(x1t_aug[0:128]),
                "x1t1": np.ascontiguousarray(x1t_aug[128:256]),
                "x1t2": np.ascontiguousarray(x1t_aug[256 : F + 1]),
                "ident": ident,
            }
        )
    return in_maps


def _run_spmd(x1, conv_w, conv_b, trace=False, **run_kwargs):
    in_maps = _prep_inputs(x1, conv_w, conv_b)
    nc = _get_program()
    res = bass_utils.run_bass_kernel_spmd(
        nc, in_maps, core_ids=list(range(N_CORES)), trace=trace, **run_kwargs
    )

    out = np.empty((B, N, N), dtype=np.float32)
    for k in range(N_CORES):
        b, m = divmod(k, BLOCKS_PER_BATCH)
        blk = np.asarray(res.results[k]["out"]).astype(np.float32)
        out[b, m * ROWS_PER_CORE : (m + 1) * ROWS_PER_CORE, :] = np.roll(
            blk, ROWS_PER_CORE * m, axis=0
        ).T
    return out, res


def kernel(x1, conv_w, conv_b):
    return _run_spmd(x1, conv_w, conv_b)[0]
